# revision 16
# baseline (speedup 1.0000x reference)
"""Trainium2 Bass kernel for nn_Contrast_loss (B=8192, D=256, 100 classes).

Math: with mask = -same + 0.5*(1-same) + I and same_ii = 1,
    loss = 0.5*||s||^2 - 1.5*sum_c ||g_c||^2 + sum_i ||f_i||^2
where s = sum_i f_i and g_c = sum_{i: label_i = c} f_i.

Every term decomposes over feature columns, so feat is sharded
column-wise across the 8 cores (32 columns each); the host sums the 8
partial scalars. No cross-core collective.

Layout trick: the loss is invariant to row permutations, so the host
sorts rows by label and pads every class to a fixed 128 slots (max
count for this distribution is ~105; a 256-slot variant is compiled
lazily as a fallback). Device layout per core:
    fpad[p, q, s] with partition p = 32*(c%4) + col, q = c//4, slot s.
Then every per-class sum g_c is just a DVE tensor_reduce over the
innermost (slot) axis - no labels, no one-hot, no per-chunk matmuls on
device at all. The tensor engine only does the final [1,1] partition
reduction. feat is staged as bf16 (halves DMA; ~1e-2 rel err is within
the 2e-2 gate). The feat DMA is split across both HWDGE rings
(sync + scalar engines) to overlap the ~1-2us fixed completion cost.
"""

import numpy as np
import ml_dtypes

import concourse.bacc as bacc
import concourse.bass as bass
import concourse.mybir as mybir
import concourse.tile as tile
from concourse import bass_utils

B = 8192
D = 256
N_CORES = 8
DPC = D // N_CORES          # 32 columns per core
P = 128                     # partitions
NCLS = 100
MG = 4                      # classes interleaved across partition groups
Q = NCLS // MG              # 25 class groups along free dim
LAMDA = 0.5

FP32 = mybir.dt.float32
BF16 = mybir.dt.bfloat16

# q-ranges: [0, QH) lands first (sync ring), [QH, Q) second (scalar ring)
QH = 13
# diag split: Act squares the first DMA half, DVE the second
ACT_Q = (0, QH)
DVE_Q = (QH, Q)

_CACHED = {}


def _build_nc(slots):
    nc = bacc.Bacc("TRN2", target_bir_lowering=False, debug=False,
                   num_devices=N_CORES)

    feat_d = nc.dram_tensor("feat", [P, Q * slots], BF16,
                            kind="ExternalInput")
    amat_d = nc.dram_tensor("amat", [P, DPC], BF16, kind="ExternalInput")
    out_d = nc.dram_tensor("out", [1, 1], FP32, kind="ExternalOutput")

    with tile.TileContext(nc) as tc:
        with (
            tc.tile_pool(name="big", bufs=1) as big,
            tc.tile_pool(name="small", bufs=1) as small,
            tc.tile_pool(name="psum", bufs=1, space="PSUM") as psum,
        ):
            fpad = big.tile([P, Q, slots], BF16)
            ndve = Q - QH
            sq_t = big.tile([P, ndve, slots], BF16)   # DVE diag scratch
            sqa_t = big.tile([P, QH, slots], BF16)    # Act diag scratch

            src = feat_d.rearrange("p (q s) -> p q s", q=Q)
            h1 = slice(0, QH)
            h2 = slice(QH, Q)
            amat_t = small.tile([P, DPC], BF16)
            nc.sync.dma_start(fpad[:, h1, :], src[:, h1, :])
            nc.scalar.dma_start(fpad[:, h2, :], src[:, h2, :])
            nc.gpsimd.dma_start(amat_t[:], amat_d.rearrange("p j -> p j"))

            # per-class sums: g[p, q] = sum_s fpad[p, q, s]
            g_t = small.tile([P, Q], FP32)
            nc.vector.tensor_reduce(g_t[:, h1], fpad[:, h1, :],
                                    mybir.AxisListType.X, mybir.AluOpType.add)
            nc.vector.tensor_reduce(g_t[:, h2], fpad[:, h2, :],
                                    mybir.AxisListType.X, mybir.AluOpType.add)

            # diag term sum f^2 (padded zeros contribute nothing)
            dacc_t = small.tile([P, 1], FP32)
            nc.scalar.activation(
                sqa_t[:], fpad[:, h1, :],
                mybir.ActivationFunctionType.Square,
                accum_out=dacc_t[:, 0:1])
            dd_t = small.tile([P, 1], FP32)
            nc.vector.tensor_mul(sq_t[:], fpad[:, h2, :], fpad[:, h2, :])
            nc.vector.tensor_reduce(dd_t[:], sq_t[:],
                                    mybir.AxisListType.XY,
                                    mybir.AluOpType.add)

            # epilogue
            sqg_t = small.tile([P, Q], FP32)
            qsum_t = small.tile([P, 1], FP32)
            sg_t = small.tile([P, 1], FP32)
            nc.vector.tensor_mul(sqg_t[:], g_t[:], g_t[:])
            nc.vector.tensor_reduce(qsum_t[:], sqg_t[:],
                                    mybir.AxisListType.X, mybir.AluOpType.add)
            nc.vector.tensor_reduce(sg_t[:], g_t[:],
                                    mybir.AxisListType.X, mybir.AluOpType.add)
            # s_col[j] = sum_m sg[32m + j] = (A^T sg)[j], A[p,j] = (p%32==j)
            # sg is carried as an exact bf16 hi/lo pair through the PE
            sgb_t = small.tile([P, 2], BF16)
            lo_t = small.tile([P, 1], FP32)
            nc.vector.tensor_copy(sgb_t[:, 0:1], sg_t[:])
            nc.vector.tensor_sub(lo_t[:], sg_t[:], sgb_t[:, 0:1])
            nc.vector.tensor_copy(sgb_t[:, 1:2], lo_t[:])
            psum_s = psum.tile([DPC, 2], FP32)
            nc.tensor.matmul(psum_s[:], amat_t[:], sgb_t[:],
                             start=True, stop=True)
            sM_t = small.tile([DPC, 1], FP32)
            nc.vector.tensor_reduce(sM_t[:], psum_s[:],
                                    mybir.AxisListType.X, mybir.AluOpType.add)
            s2_t = small.tile([DPC, 1], FP32)
            nc.vector.tensor_mul(s2_t[:], sM_t[:], sM_t[:])
            nc.vector.tensor_scalar_mul(s2_t[:], s2_t[:], LAMDA)

            comb_t = small.tile([P, 1], FP32)
            nc.vector.tensor_add(comb_t[:], dacc_t[:], dd_t[:])
            nc.vector.scalar_tensor_tensor(
                comb_t[:], qsum_t[:], -(1.0 + LAMDA), comb_t[:],
                mybir.AluOpType.mult, mybir.AluOpType.add)
            nc.vector.tensor_add(comb_t[0:DPC, :], comb_t[0:DPC, :], s2_t[:])

            ones_t = small.tile([P, 1], FP32)
            nc.vector.memset(ones_t[:], 1.0)
            psum_out = psum.tile([1, 1], FP32)
            nc.tensor.matmul(psum_out[:], comb_t[:], ones_t[:],
                             start=True, stop=True)
            res_t = small.tile([1, 1], FP32)
            nc.scalar.copy(res_t[:], psum_out[:])
            nc.sync.dma_start(out_d[:], res_t[:])

    nc.compile()
    return nc


def _get_nc(slots):
    if slots not in _CACHED:
        _CACHED[slots] = _build_nc(slots)
    return _CACHED[slots]


def _prep(feat, label):
    feat = np.asarray(feat, dtype=np.float32)
    label = np.asarray(label).astype(np.int64).ravel()
    cnt = np.bincount(label, minlength=NCLS)
    slots = 128 if cnt.max() <= 128 else 256

    order = np.argsort(label, kind="stable")
    lab_s = label[order]
    start = np.zeros(NCLS, dtype=np.int64)
    start[1:] = np.cumsum(cnt)[:-1]
    pos = np.arange(B) - start[lab_s]          # slot within class

    padded = np.zeros((NCLS, slots, D), dtype=np.float32)
    padded[lab_s, pos, :] = feat[order]
    # device layout: [core, p = 32*m + j, q*slots + s]
    pr = padded.reshape(Q, MG, slots, N_CORES, DPC)
    dev = pr.transpose(3, 1, 4, 0, 2).reshape(N_CORES, P, Q * slots)
    return np.ascontiguousarray(dev.astype(ml_dtypes.bfloat16)), slots


_AMAT = np.ascontiguousarray(
    (np.arange(P)[:, None] % DPC == np.arange(DPC)[None, :])
    .astype(ml_dtypes.bfloat16))


def kernel(feat, label, _trace=False):
    dev, slots = _prep(feat, label)
    nc = _get_nc(slots)
    in_maps = [{"feat": dev[m], "amat": _AMAT} for m in range(N_CORES)]
    res = bass_utils.run_bass_kernel_spmd(
        nc, in_maps, core_ids=list(range(N_CORES)), trace=_trace)
    total = np.float64(0.0)
    for r in res.results:
        total += np.float64(r["out"][0, 0])
    out = np.float32(total)
    if _trace:
        return out, res
    return out


# revision 17
# speedup vs baseline: 1.2456x; 1.2456x over previous
"""Trainium2 Bass kernel for nn_Contrast_loss (B=8192, D=256, 100 classes).

Math: with mask = -same + 0.5*(1-same) + I and same_ii = 1,
    loss = 0.5*||s||^2 - 1.5*sum_c ||g_c||^2 + sum_i ||f_i||^2
where s = sum_i f_i and g_c = sum_{i: label_i = c} f_i.

Every term decomposes over feature columns, so feat is sharded
column-wise across the 8 cores (32 columns each); the host sums the 8
partial scalars. No cross-core collective.

Layout trick: the loss is invariant to row permutations, so the host
sorts rows by label and pads every class to a fixed slot count (112 for
this distribution; 128/256 variants compile lazily as fallbacks).
Device layout per core:
    fpad[p, q, s] with partition p = 32*(c%4) + col, q = c//4, slot s.
Every per-class sum g_c is then one lane-local slice of a DVE
tensor_reduce over the innermost (slot) axis - no labels, no one-hot,
no per-chunk matmuls. The tensor engine only reduces across partitions
at the very end (via a tiny host-supplied block-identity A with
A[p,j] = (p%32 == j), and an all-ones vector).

feat is staged as bf16 (halves DMA; ~1.1e-2 rel err is within the 2e-2
gate) and transferred as 4 pieces alternating between the two HWDGE
rings (sync/scalar) so squares and reduces pipeline behind the DMA.
"""

import numpy as np
import ml_dtypes

import concourse.bacc as bacc
import concourse.bass as bass
import concourse.mybir as mybir
import concourse.tile as tile
from concourse import bass_utils

B = 8192
D = 256
N_CORES = 8
DPC = D // N_CORES          # 32 columns per core
P = 128                     # partitions
NCLS = 100
MG = 4                      # classes interleaved across partition groups
Q = NCLS // MG              # 25 class groups along free dim
LAMDA = 0.5

FP32 = mybir.dt.float32
BF16 = mybir.dt.bfloat16

# 4 DMA pieces: [0,7) and [13,19) on the sync ring, [7,13) and [19,25)
# on the scalar ring; pieces complete in this interleaved order
PIECES = ((0, 7), (7, 13), (13, 19), (19, 25))
RINGS = ("sync", "scalar", "sync", "scalar")

_CACHED = {}


def _build_nc(slots):
    nc = bacc.Bacc("TRN2", target_bir_lowering=False, debug=False,
                   num_devices=N_CORES)

    feat_d = nc.dram_tensor("feat", [P, Q * slots], BF16,
                            kind="ExternalInput")
    amat_d = nc.dram_tensor("amat", [P, DPC], BF16, kind="ExternalInput")
    out_d = nc.dram_tensor("out", [1, 1], FP32, kind="ExternalOutput")

    with tile.TileContext(nc) as tc:
        with (
            tc.tile_pool(name="big", bufs=1) as big,
            tc.tile_pool(name="small", bufs=1) as small,
            tc.tile_pool(name="psum", bufs=1, space="PSUM") as psum,
        ):
            fpad = big.tile([P, Q, slots], BF16)
            sqa_t = big.tile([P, 7, slots], BF16)   # Act square scratch
            amat_t = small.tile([P, DPC], BF16)
            g_t = small.tile([P, Q], FP32)
            dacc_t = small.tile([P, len(PIECES)], FP32)

            src = feat_d.rearrange("p (q s) -> p q s", q=Q)
            for (q0, q1), ring in zip(PIECES, RINGS):
                eng = nc.sync if ring == "sync" else nc.scalar
                eng.dma_start(fpad[:, q0:q1, :], src[:, q0:q1, :])
            nc.sync.dma_start(amat_t[:], amat_d.rearrange("p j -> p j"))

            # pipelined per piece: Act squares (diag term), DVE class sums
            for i, (q0, q1) in enumerate(PIECES):
                nc.scalar.activation(
                    sqa_t[:, 0:q1 - q0, :], fpad[:, q0:q1, :],
                    mybir.ActivationFunctionType.Square,
                    accum_out=dacc_t[:, i:i + 1])
                nc.vector.tensor_reduce(g_t[:, q0:q1], fpad[:, q0:q1, :],
                                        mybir.AxisListType.X,
                                        mybir.AluOpType.add)

            # epilogue (all fp32, small)
            sg_t = small.tile([P, 1], FP32)
            nc.vector.tensor_reduce(sg_t[:], g_t[:],
                                    mybir.AxisListType.X, mybir.AluOpType.add)
            # sg as exact bf16 hi/lo pair for the PE partition reduce
            sgb_t = small.tile([P, 2], BF16)
            lo_t = small.tile([P, 1], FP32)
            nc.vector.tensor_copy(sgb_t[:, 0:1], sg_t[:])
            nc.vector.tensor_sub(lo_t[:], sg_t[:], sgb_t[:, 0:1])
            nc.vector.tensor_copy(sgb_t[:, 1:2], lo_t[:])
            psum_s = psum.tile([DPC, 2], FP32)
            nc.tensor.matmul(psum_s[:], amat_t[:], sgb_t[:],
                             start=True, stop=True)

            sqg_t = small.tile([P, Q], FP32)
            qsum_t = small.tile([P, 1], FP32)
            nc.vector.tensor_mul(sqg_t[:], g_t[:], g_t[:])
            nc.vector.tensor_reduce(qsum_t[:], sqg_t[:],
                                    mybir.AxisListType.X, mybir.AluOpType.add)
            d1_t = small.tile([P, 1], FP32)
            nc.vector.tensor_reduce(d1_t[:], dacc_t[:],
                                    mybir.AxisListType.X, mybir.AluOpType.add)
            comb_t = small.tile([P, 1], FP32)
            nc.vector.scalar_tensor_tensor(
                comb_t[:], qsum_t[:], -(1.0 + LAMDA), d1_t[:],
                mybir.AluOpType.mult, mybir.AluOpType.add)

            # s_col[j] = (A^T sg)[j]; add 0.5*s_col^2 into comb rows 0:DPC
            sM_t = small.tile([DPC, 1], FP32)
            nc.vector.tensor_reduce(sM_t[:], psum_s[:],
                                    mybir.AxisListType.X, mybir.AluOpType.add)
            s2_t = small.tile([DPC, 1], FP32)
            nc.vector.scalar_tensor_tensor(
                s2_t[:], sM_t[:], LAMDA, sM_t[:],
                mybir.AluOpType.mult, mybir.AluOpType.mult)
            nc.vector.tensor_add(comb_t[0:DPC, :], comb_t[0:DPC, :], s2_t[:])

            ones_t = small.tile([P, 1], FP32)
            nc.vector.memset(ones_t[:], 1.0)
            psum_out = psum.tile([1, 1], FP32)
            nc.tensor.matmul(psum_out[:], comb_t[:], ones_t[:],
                             start=True, stop=True)
            res_t = small.tile([1, 1], FP32)
            nc.scalar.copy(res_t[:], psum_out[:])
            nc.sync.dma_start(out_d[:], res_t[:])

    nc.compile()
    return nc


def _get_nc(slots):
    if slots not in _CACHED:
        _CACHED[slots] = _build_nc(slots)
    return _CACHED[slots]


def _prep(feat, label):
    feat = np.asarray(feat, dtype=np.float32)
    label = np.asarray(label).astype(np.int64).ravel()
    cnt = np.bincount(label, minlength=NCLS)
    mx = cnt.max()
    slots = 112 if mx <= 112 else (128 if mx <= 128 else 256)

    order = np.argsort(label, kind="stable")
    lab_s = label[order]
    start = np.zeros(NCLS, dtype=np.int64)
    start[1:] = np.cumsum(cnt)[:-1]
    pos = np.arange(B) - start[lab_s]          # slot within class

    padded = np.zeros((NCLS, slots, D), dtype=np.float32)
    padded[lab_s, pos, :] = feat[order]
    # device layout: [core, p = 32*m + j, q*slots + s]
    pr = padded.reshape(Q, MG, slots, N_CORES, DPC)
    dev = pr.transpose(3, 1, 4, 0, 2).reshape(N_CORES, P, Q * slots)
    return np.ascontiguousarray(dev.astype(ml_dtypes.bfloat16)), slots


_AMAT = np.ascontiguousarray(
    (np.arange(P)[:, None] % DPC == np.arange(DPC)[None, :])
    .astype(ml_dtypes.bfloat16))


def kernel(feat, label, _trace=False):
    dev, slots = _prep(feat, label)
    nc = _get_nc(slots)
    in_maps = [{"feat": dev[m], "amat": _AMAT} for m in range(N_CORES)]
    res = bass_utils.run_bass_kernel_spmd(
        nc, in_maps, core_ids=list(range(N_CORES)), trace=_trace)
    total = np.float64(0.0)
    for r in res.results:
        total += np.float64(r["out"][0, 0])
    out = np.float32(total)
    if _trace:
        return out, res
    return out


# revision 22
# speedup vs baseline: 1.2729x; 1.0219x over previous
"""Trainium2 Bass kernel for nn_Contrast_loss (B=8192, D=256, 100 classes).

Math: with mask = -same + 0.5*(1-same) + I and same_ii = 1,
    loss = 0.5*||s||^2 - 1.5*sum_c ||g_c||^2 + sum_i ||f_i||^2
where s = sum_i f_i and g_c = sum_{i: label_i = c} f_i.

Every term decomposes over feature columns, so feat is sharded
column-wise across the 8 cores (32 columns each); the host sums the 8
partial scalars. No cross-core collective.

Layout trick: the loss is invariant to row permutations, so the host
sorts rows by label and pads every class to a fixed slot count (112 for
this distribution; 128/256 variants compile lazily as fallbacks).
Device layout per core:
    fpad[p, q, s] with partition p = 32*(c%4) + col, q = c//4, slot s.
Every per-class sum g_c is then one lane-local slice of a DVE
tensor_reduce over the innermost (slot) axis - no labels, no one-hot,
no per-chunk matmuls. The tensor engine only reduces across partitions
at the very end (via a tiny host-supplied block-identity A with
A[p,j] = (p%32 == j), and an all-ones vector).

feat is staged as bf16 (halves DMA; ~1.1e-2 rel err is within the 2e-2
gate) and transferred as 4 pieces alternating between the two HWDGE
rings (sync/scalar) so squares and reduces pipeline behind the DMA.
"""

import numpy as np
import ml_dtypes

import concourse.bacc as bacc
import concourse.bass as bass
import concourse.mybir as mybir
import concourse.tile as tile
from concourse import bass_utils

B = 8192
D = 256
N_CORES = 8
DPC = D // N_CORES          # 32 columns per core
P = 128                     # partitions
NCLS = 100
MG = 4                      # classes interleaved across partition groups
Q = NCLS // MG              # 25 class groups along free dim
LAMDA = 0.5

FP32 = mybir.dt.float32
BF16 = mybir.dt.bfloat16

# DMA pieces alternate between the two HWDGE rings; the first is small
# so the compute pipeline primes as early as possible
PIECES = ((0, 3), (3, 8), (8, 14), (14, 20), (20, 25))
RINGS = ("sync", "scalar", "sync", "scalar", "sync")

_CACHED = {}


def _build_nc(slots):
    nc = bacc.Bacc("TRN2", target_bir_lowering=False, debug=False,
                   num_devices=N_CORES)

    feat_d = nc.dram_tensor("feat", [P, Q * slots], BF16,
                            kind="ExternalInput")
    amat_d = nc.dram_tensor("amat", [P, DPC], BF16, kind="ExternalInput")
    out_d = nc.dram_tensor("out", [1, 1], FP32, kind="ExternalOutput")

    with tile.TileContext(nc) as tc:
        with (
            tc.tile_pool(name="big", bufs=1) as big,
            tc.tile_pool(name="small", bufs=1) as small,
            tc.tile_pool(name="psum", bufs=1, space="PSUM") as psum,
        ):
            fpad = big.tile([P, Q, slots], BF16)
            sqa_t = big.tile([P, max(q1 - q0 for q0, q1 in PIECES), slots],
                             BF16)                  # Act square scratch
            amat_t = small.tile([P, DPC], BF16)
            g_t = small.tile([P, Q], FP32)
            dacc_t = small.tile([P, len(PIECES)], FP32)

            src = feat_d.rearrange("p (q s) -> p q s", q=Q)
            for (q0, q1), ring in zip(PIECES, RINGS):
                eng = nc.sync if ring == "sync" else nc.scalar
                eng.dma_start(fpad[:, q0:q1, :], src[:, q0:q1, :])
            nc.scalar.dma_start(amat_t[:], amat_d.rearrange("p j -> p j"))

            # pipelined per piece: Act squares (diag term), DVE class sums
            for i, (q0, q1) in enumerate(PIECES):
                nc.scalar.activation(
                    sqa_t[:, 0:q1 - q0, :], fpad[:, q0:q1, :],
                    mybir.ActivationFunctionType.Square,
                    accum_out=dacc_t[:, i:i + 1])
                nc.vector.tensor_reduce(g_t[:, q0:q1], fpad[:, q0:q1, :],
                                        mybir.AxisListType.X,
                                        mybir.AluOpType.add)

            # epilogue (all fp32, small)
            sg_t = small.tile([P, 1], FP32)
            nc.vector.tensor_reduce(sg_t[:], g_t[:],
                                    mybir.AxisListType.X, mybir.AluOpType.add)
            # sg as exact bf16 hi/lo pair for the PE partition reduce
            sgb_t = small.tile([P, 2], BF16)
            lo_t = small.tile([P, 1], FP32)
            nc.vector.tensor_copy(sgb_t[:, 0:1], sg_t[:])
            nc.vector.tensor_sub(lo_t[:], sg_t[:], sgb_t[:, 0:1])
            nc.vector.tensor_copy(sgb_t[:, 1:2], lo_t[:])
            psum_s = psum.tile([DPC, 2], FP32)
            nc.tensor.matmul(psum_s[:], amat_t[:], sgb_t[:],
                             start=True, stop=True)

            # qsum = sum(-1.5 * g^2) fused in one DVE pass
            sqg_t = small.tile([P, Q], FP32)
            qsum_t = small.tile([P, 1], FP32)
            nc.vector.affine_mul_reduce(sqg_t[:], qsum_t[:], g_t[:], g_t[:],
                                        -(1.0 + LAMDA), 0.0)
            d1_t = small.tile([P, 1], FP32)
            nc.vector.tensor_reduce(d1_t[:], dacc_t[:],
                                    mybir.AxisListType.X, mybir.AluOpType.add)
            comb_t = small.tile([P, 1], FP32)
            nc.vector.tensor_add(comb_t[:], qsum_t[:], d1_t[:])

            # s_col[j] = (A^T sg)[j]; add 0.5*s_col^2 into comb rows 0:DPC
            sM_t = small.tile([DPC, 1], FP32)
            nc.vector.tensor_reduce(sM_t[:], psum_s[:],
                                    mybir.AxisListType.X, mybir.AluOpType.add)
            s2_t = small.tile([DPC, 1], FP32)
            nc.vector.scalar_tensor_tensor(
                s2_t[:], sM_t[:], LAMDA, sM_t[:],
                mybir.AluOpType.mult, mybir.AluOpType.mult)
            nc.vector.tensor_add(comb_t[0:DPC, :], comb_t[0:DPC, :], s2_t[:])

            ones_t = small.tile([P, 1], FP32)
            nc.vector.memset(ones_t[:], 1.0)
            psum_out = psum.tile([1, 1], FP32)
            nc.tensor.matmul(psum_out[:], comb_t[:], ones_t[:],
                             start=True, stop=True)
            res_t = small.tile([1, 1], FP32)
            nc.vector.tensor_copy(res_t[:], psum_out[:])
            nc.sync.dma_start(out_d[:], res_t[:])

    nc.compile()
    return nc


def _get_nc(slots):
    if slots not in _CACHED:
        _CACHED[slots] = _build_nc(slots)
    return _CACHED[slots]


def _prep(feat, label):
    feat = np.asarray(feat, dtype=np.float32)
    label = np.asarray(label).astype(np.int64).ravel()
    cnt = np.bincount(label, minlength=NCLS)
    mx = cnt.max()
    slots = 112 if mx <= 112 else (128 if mx <= 128 else 256)

    order = np.argsort(label, kind="stable")
    lab_s = label[order]
    start = np.zeros(NCLS, dtype=np.int64)
    start[1:] = np.cumsum(cnt)[:-1]
    pos = np.arange(B) - start[lab_s]          # slot within class

    padded = np.zeros((NCLS, slots, D), dtype=np.float32)
    padded[lab_s, pos, :] = feat[order]
    # device layout: [core, p = 32*m + j, q*slots + s]
    pr = padded.reshape(Q, MG, slots, N_CORES, DPC)
    dev = pr.transpose(3, 1, 4, 0, 2).reshape(N_CORES, P, Q * slots)
    return np.ascontiguousarray(dev.astype(ml_dtypes.bfloat16)), slots


_AMAT = np.ascontiguousarray(
    (np.arange(P)[:, None] % DPC == np.arange(DPC)[None, :])
    .astype(ml_dtypes.bfloat16))


def kernel(feat, label, _trace=False):
    dev, slots = _prep(feat, label)
    nc = _get_nc(slots)
    in_maps = [{"feat": dev[m], "amat": _AMAT} for m in range(N_CORES)]
    res = bass_utils.run_bass_kernel_spmd(
        nc, in_maps, core_ids=list(range(N_CORES)), trace=_trace)
    total = np.float64(0.0)
    for r in res.results:
        total += np.float64(r["out"][0, 0])
    out = np.float32(total)
    if _trace:
        return out, res
    return out


# revision 30
# speedup vs baseline: 1.2945x; 1.0169x over previous
"""Trainium2 Bass kernel for nn_Contrast_loss (B=8192, D=256, 100 classes).

Math: with mask = -same + 0.5*(1-same) + I and same_ii = 1,
    loss = 0.5*||s||^2 - 1.5*sum_c ||g_c||^2 + sum_i ||f_i||^2
where s = sum_i f_i and g_c = sum_{i: label_i = c} f_i.

Every term decomposes over feature columns, so feat is sharded
column-wise across the 8 cores (32 columns each); the host sums the 8
partial scalars. No cross-core collective.

Layout trick: the loss is invariant to row permutations, so the host
sorts rows by label and pads every class to a fixed slot count (112 for
this distribution; 128/256 variants compile lazily as fallbacks).
Device layout per core:
    fpad[p, q, s] with partition p = 32*(c%4) + col, q = c//4, slot s.
Every per-class sum g_c is then one lane-local slice of a DVE
tensor_reduce over the innermost (slot) axis - no labels, no one-hot,
no per-chunk matmuls. The tensor engine only reduces across partitions
at the very end (via a tiny host-supplied block-identity A with
A[p,j] = (p%32 == j), and an all-ones vector).

feat is staged as bf16 (halves DMA; ~1.1e-2 rel err is within the 2e-2
gate) and transferred as 4 pieces alternating between the two HWDGE
rings (sync/scalar) so squares and reduces pipeline behind the DMA.
"""

import numpy as np
import ml_dtypes

import concourse.bacc as bacc
import concourse.bass as bass
import concourse.mybir as mybir
import concourse.tile as tile
from concourse import bass_utils

B = 8192
D = 256
N_CORES = 8
DPC = D // N_CORES          # 32 columns per core
P = 128                     # partitions
NCLS = 100
MG = 4                      # classes interleaved across partition groups
Q = NCLS // MG              # 25 class groups along free dim
LAMDA = 0.5

FP32 = mybir.dt.float32
BF16 = mybir.dt.bfloat16

# DMA pieces alternate between the two HWDGE rings; the first is small
# so the compute pipeline primes as early as possible
PIECES = ((0, 3), (3, 8), (8, 14), (14, 20), (20, 25))
RINGS = ("sync", "scalar", "sync", "scalar", "sync")

_CACHED = {}


def _build_nc(slots):
    nc = bacc.Bacc("TRN2", target_bir_lowering=False, debug=False,
                   num_devices=N_CORES)

    feat_d = nc.dram_tensor("feat", [P, Q * slots], BF16,
                            kind="ExternalInput")
    amat_d = nc.dram_tensor("amat", [P, DPC], BF16, kind="ExternalInput")
    # out cols 0..5: per-partition loss pieces (-1.5*sum g^2, per-piece
    # sum f^2); rows 0:DPC of cols 6:8: hi/lo parts of the column sums
    # s_j. The host finishes this tiny (128x8) combine.
    NOUT = 8
    out_d = nc.dram_tensor("out", [P, NOUT], FP32, kind="ExternalOutput")

    with tile.TileContext(nc) as tc:
        with (
            tc.tile_pool(name="big", bufs=1) as big,
            tc.tile_pool(name="small", bufs=1) as small,
            tc.tile_pool(name="psum", bufs=1, space="PSUM") as psum,
        ):
            fpad = big.tile([P, Q, slots], BF16)
            sqa_t = big.tile([P, max(q1 - q0 for q0, q1 in PIECES), slots],
                             BF16)                  # Act square scratch
            amat_t = small.tile([P, DPC], BF16)
            g_t = small.tile([P, Q], FP32)
            # col 0: -1.5*sum g^2 (amr); cols 1..5: per-piece sum f^2
            # (Act); [0:DPC, 6:8]: s_j hi/lo from the PE. DMA'd out whole.
            big6 = small.tile([P, NOUT], FP32)
            nc.vector.memset(big6[:], 0.0)

            src = feat_d.rearrange("p (q s) -> p q s", q=Q)
            for (q0, q1), ring in zip(PIECES, RINGS):
                eng = nc.sync if ring == "sync" else nc.scalar
                eng.dma_start(fpad[:, q0:q1, :], src[:, q0:q1, :])
            nc.scalar.dma_start(amat_t[:], amat_d.rearrange("p j -> p j"))

            # pipelined per piece: Act squares (diag term), DVE class sums
            for i, (q0, q1) in enumerate(PIECES):
                nc.scalar.activation(
                    sqa_t[:, 0:q1 - q0, :], fpad[:, q0:q1, :],
                    mybir.ActivationFunctionType.Square,
                    accum_out=big6[:, 1 + i:2 + i])
                nc.vector.tensor_reduce(g_t[:, q0:q1], fpad[:, q0:q1, :],
                                        mybir.AxisListType.X,
                                        mybir.AluOpType.add)

            # epilogue (all fp32, small)
            sg_t = small.tile([P, 1], FP32)
            nc.vector.tensor_reduce(sg_t[:], g_t[:],
                                    mybir.AxisListType.X, mybir.AluOpType.add)
            # sg as exact bf16 hi/lo pair for the PE partition reduce
            sgb_t = small.tile([P, 2], BF16)
            lo_t = small.tile([P, 1], FP32)
            nc.vector.tensor_copy(sgb_t[:, 0:1], sg_t[:])
            nc.vector.tensor_sub(lo_t[:], sg_t[:], sgb_t[:, 0:1])
            nc.vector.tensor_copy(sgb_t[:, 1:2], lo_t[:])
            psum_s = psum.tile([DPC, 2], FP32)
            nc.tensor.matmul(psum_s[:], amat_t[:], sgb_t[:],
                             start=True, stop=True)

            # big6 col 0 = -1.5*sum_q g^2, fused in one DVE pass
            sqg_t = small.tile([P, Q], FP32)
            nc.vector.affine_mul_reduce(sqg_t[:], big6[:, 0:1], g_t[:], g_t[:],
                                        -(1.0 + LAMDA), 0.0)
            nc.vector.tensor_copy(big6[0:DPC, 6:8], psum_s[:])
            nc.sync.dma_start(out_d[:], big6[:])

    nc.compile()
    return nc


def _get_nc(slots):
    if slots not in _CACHED:
        _CACHED[slots] = _build_nc(slots)
    return _CACHED[slots]


def _prep(feat, label):
    feat = np.asarray(feat, dtype=np.float32)
    label = np.asarray(label).astype(np.int64).ravel()
    cnt = np.bincount(label, minlength=NCLS)
    mx = cnt.max()
    slots = 112 if mx <= 112 else (128 if mx <= 128 else 256)

    order = np.argsort(label, kind="stable")
    lab_s = label[order]
    start = np.zeros(NCLS, dtype=np.int64)
    start[1:] = np.cumsum(cnt)[:-1]
    pos = np.arange(B) - start[lab_s]          # slot within class

    padded = np.zeros((NCLS, slots, D), dtype=np.float32)
    padded[lab_s, pos, :] = feat[order]
    # device layout: [core, p = 32*m + j, q*slots + s]
    pr = padded.reshape(Q, MG, slots, N_CORES, DPC)
    dev = pr.transpose(3, 1, 4, 0, 2).reshape(N_CORES, P, Q * slots)
    return np.ascontiguousarray(dev.astype(ml_dtypes.bfloat16)), slots


_AMAT = np.ascontiguousarray(
    (np.arange(P)[:, None] % DPC == np.arange(DPC)[None, :])
    .astype(ml_dtypes.bfloat16))


def kernel(feat, label, _trace=False):
    dev, slots = _prep(feat, label)
    nc = _get_nc(slots)
    in_maps = [{"feat": dev[m], "amat": _AMAT} for m in range(N_CORES)]
    res = bass_utils.run_bass_kernel_spmd(
        nc, in_maps, core_ids=list(range(N_CORES)), trace=_trace)
    total = np.float64(0.0)
    for r in res.results:
        o = np.asarray(r["out"], dtype=np.float64)
        total += o[:, 0:6].sum()
        sj = o[0:DPC, 6] + o[0:DPC, 7]
        total += LAMDA * np.square(sj).sum()
    out = np.float32(total)
    if _trace:
        return out, res
    return out


# revision 31
# speedup vs baseline: 1.3071x; 1.0098x over previous
"""Trainium2 Bass kernel for nn_Contrast_loss (B=8192, D=256, 100 classes).

Math: with mask = -same + 0.5*(1-same) + I and same_ii = 1,
    loss = 0.5*||s||^2 - 1.5*sum_c ||g_c||^2 + sum_i ||f_i||^2
where s = sum_i f_i and g_c = sum_{i: label_i = c} f_i.

Every term decomposes over feature columns, so feat is sharded
column-wise across the 8 cores (32 columns each); the host sums the 8
partial scalars. No cross-core collective.

Layout trick: the loss is invariant to row permutations, so the host
sorts rows by label and pads every class to a fixed slot count (112 for
this distribution; 128/256 variants compile lazily as fallbacks).
Device layout per core:
    fpad[p, q, s] with partition p = 32*(c%4) + col, q = c//4, slot s.
Every per-class sum g_c is then one lane-local slice of a DVE
tensor_reduce over the innermost (slot) axis - no labels, no one-hot,
no per-chunk matmuls. The tensor engine only reduces across partitions
at the very end (via a tiny host-supplied block-identity A with
A[p,j] = (p%32 == j), and an all-ones vector).

feat is staged as bf16 (halves DMA; ~1.1e-2 rel err is within the 2e-2
gate) and transferred as 4 pieces alternating between the two HWDGE
rings (sync/scalar) so squares and reduces pipeline behind the DMA.
"""

import numpy as np
import ml_dtypes

import concourse.bacc as bacc
import concourse.bass as bass
import concourse.mybir as mybir
import concourse.tile as tile
from concourse import bass_utils

B = 8192
D = 256
N_CORES = 8
DPC = D // N_CORES          # 32 columns per core
P = 128                     # partitions
NCLS = 100
MG = 4                      # classes interleaved across partition groups
Q = NCLS // MG              # 25 class groups along free dim
LAMDA = 0.5

FP32 = mybir.dt.float32
BF16 = mybir.dt.bfloat16

# DMA pieces alternate between the two HWDGE rings; the first is small
# so the compute pipeline primes as early as possible
PIECES = ((0, 2), (2, 7), (7, 13), (13, 19), (19, 25))
RINGS = ("sync", "scalar", "sync", "scalar", "sync")

_CACHED = {}


def _build_nc(slots):
    nc = bacc.Bacc("TRN2", target_bir_lowering=False, debug=False,
                   num_devices=N_CORES)

    feat_d = nc.dram_tensor("feat", [P, Q * slots], BF16,
                            kind="ExternalInput")
    amat_d = nc.dram_tensor("amat", [P, DPC], BF16, kind="ExternalInput")
    # out cols 0..5: per-partition loss pieces (-1.5*sum g^2, per-piece
    # sum f^2); rows 0:DPC of cols 6:8: hi/lo parts of the column sums
    # s_j. The host finishes this tiny (128x8) combine.
    NOUT = 8
    out_d = nc.dram_tensor("out", [P, NOUT], FP32, kind="ExternalOutput")

    with tile.TileContext(nc) as tc:
        with (
            tc.tile_pool(name="big", bufs=1) as big,
            tc.tile_pool(name="small", bufs=1) as small,
            tc.tile_pool(name="psum", bufs=1, space="PSUM") as psum,
        ):
            fpad = big.tile([P, Q, slots], BF16)
            sqa_t = big.tile([P, max(q1 - q0 for q0, q1 in PIECES), slots],
                             BF16)                  # Act square scratch
            amat_t = small.tile([P, DPC], BF16)
            g_t = small.tile([P, Q], FP32)
            # col 0: -1.5*sum g^2 (amr); cols 1..5: per-piece sum f^2
            # (Act); [0:DPC, 6:8]: s_j hi/lo from the PE. DMA'd out whole.
            big6 = small.tile([P, NOUT], FP32)
            nc.vector.memset(big6[:], 0.0)

            src = feat_d.rearrange("p (q s) -> p q s", q=Q)
            for (q0, q1), ring in zip(PIECES, RINGS):
                eng = nc.sync if ring == "sync" else nc.scalar
                eng.dma_start(fpad[:, q0:q1, :], src[:, q0:q1, :])
            nc.scalar.dma_start(amat_t[:], amat_d.rearrange("p j -> p j"))

            # pipelined per piece: Act squares (diag term), DVE class sums
            for i, (q0, q1) in enumerate(PIECES):
                nc.scalar.activation(
                    sqa_t[:, 0:q1 - q0, :], fpad[:, q0:q1, :],
                    mybir.ActivationFunctionType.Square,
                    accum_out=big6[:, 1 + i:2 + i])
                nc.vector.tensor_reduce(g_t[:, q0:q1], fpad[:, q0:q1, :],
                                        mybir.AxisListType.X,
                                        mybir.AluOpType.add)

            # epilogue (all fp32, small)
            sg_t = small.tile([P, 1], FP32)
            nc.vector.tensor_reduce(sg_t[:], g_t[:],
                                    mybir.AxisListType.X, mybir.AluOpType.add)
            # sg as exact bf16 hi/lo pair for the PE partition reduce
            sgb_t = small.tile([P, 2], BF16)
            lo_t = small.tile([P, 1], FP32)
            nc.vector.tensor_copy(sgb_t[:, 0:1], sg_t[:])
            nc.vector.tensor_sub(lo_t[:], sg_t[:], sgb_t[:, 0:1])
            nc.vector.tensor_copy(sgb_t[:, 1:2], lo_t[:])
            psum_s = psum.tile([DPC, 2], FP32)
            nc.tensor.matmul(psum_s[:], amat_t[:], sgb_t[:],
                             start=True, stop=True)

            # big6 col 0 = -1.5*sum_q g^2, fused in one DVE pass
            sqg_t = small.tile([P, Q], FP32)
            nc.vector.affine_mul_reduce(sqg_t[:], big6[:, 0:1], g_t[:], g_t[:],
                                        -(1.0 + LAMDA), 0.0)
            nc.vector.tensor_copy(big6[0:DPC, 6:8], psum_s[:])
            nc.sync.dma_start(out_d[:], big6[:])

    nc.compile()
    return nc


def _get_nc(slots):
    if slots not in _CACHED:
        _CACHED[slots] = _build_nc(slots)
    return _CACHED[slots]


def _prep(feat, label):
    feat = np.asarray(feat, dtype=np.float32)
    label = np.asarray(label).astype(np.int64).ravel()
    cnt = np.bincount(label, minlength=NCLS)
    mx = cnt.max()
    slots = 112 if mx <= 112 else (128 if mx <= 128 else 256)

    order = np.argsort(label, kind="stable")
    lab_s = label[order]
    start = np.zeros(NCLS, dtype=np.int64)
    start[1:] = np.cumsum(cnt)[:-1]
    pos = np.arange(B) - start[lab_s]          # slot within class

    padded = np.zeros((NCLS, slots, D), dtype=np.float32)
    padded[lab_s, pos, :] = feat[order]
    # device layout: [core, p = 32*m + j, q*slots + s]
    pr = padded.reshape(Q, MG, slots, N_CORES, DPC)
    dev = pr.transpose(3, 1, 4, 0, 2).reshape(N_CORES, P, Q * slots)
    return np.ascontiguousarray(dev.astype(ml_dtypes.bfloat16)), slots


_AMAT = np.ascontiguousarray(
    (np.arange(P)[:, None] % DPC == np.arange(DPC)[None, :])
    .astype(ml_dtypes.bfloat16))


def kernel(feat, label, _trace=False):
    dev, slots = _prep(feat, label)
    nc = _get_nc(slots)
    in_maps = [{"feat": dev[m], "amat": _AMAT} for m in range(N_CORES)]
    res = bass_utils.run_bass_kernel_spmd(
        nc, in_maps, core_ids=list(range(N_CORES)), trace=_trace)
    total = np.float64(0.0)
    for r in res.results:
        o = np.asarray(r["out"], dtype=np.float64)
        total += o[:, 0:6].sum()
        sj = o[0:DPC, 6] + o[0:DPC, 7]
        total += LAMDA * np.square(sj).sum()
    out = np.float32(total)
    if _trace:
        return out, res
    return out
